# revision 4
# baseline (speedup 1.0000x reference)
"""ConvLSTM (peephole, k=1) Trainium2 kernel — 8-core batch-data-parallel.

Design (v6):
  - DP-8 over batch: core j owns batch rows [8j, 8j+8). Weights replicated.
    No collectives (per-step 8-core AllGather measured ~17us serial — the
    ncfw control plane dominates; remote_dma SWDGE path crashes the axon
    worker — so cross-core exchange is off the table and DP wins).
  - Channel-major: gate preacts from weight-STATIONARY matmuls as
    [128 out-ch, batch] tiles. Zero transposes.
  - Precision: h-side SINGLE fp16 weights (hi+fp8-lo measured 2.09e-3,
    hi-only 2.46e-3 vs the 2e-2 gate — the lo pass pays 6.9us/step of
    LDWEIGHTS for nothing). x-side keeps fp16 hi + fp16*2^8 lo (x-side
    single-fp16 is what failed at 3.0e-2 historically).
  - Phase A: xg[t] = Wx@x_t + btot precomputed into DRAM.
  - Phase B: 256 serial steps; per step 256 stationary-weight matmuls
    (32 m-tiles x 8 k-tiles, N=8) as a two-pass contraction (kt 0-3 ->
    PSUM A, kt 4-7 -> PSUM B, consecutive per-m accumulation groups).
    Pass 1 of step t+1 reads only hT[:, 0:4, :] (ct-half-0 channels), so
    the half-1 gate tail of step t hides under the next LDWEIGHTS stream;
    per-step cost ~= the 256x53ns fp16 FWL weight-load floor.

Self-contained: hardcodes B=64, S=256, H=1024, 8 cores.
"""

import os
import sys

import numpy as np

sys.path.insert(0, "/opt/trn_rl_repo")

B, S, H = 64, 256, 1024
NCORES = 8
BL = B // NCORES          # 8 local batch rows
KT = H // 128             # 8 contraction k-tiles
MT = 32                   # 4 gates x 8 ct  output tiles of 128 channels
SHIFT = 2.0 ** 8          # lo-weight scale (fp16 lo, x-side)
TOK = S * BL              # 2048 tokens per core

LAST_RESULT = None


def _build_program(s_steps=S):
    import concourse.bass as bass
    import concourse.mybir as mybir
    import concourse.tile as tile
    from concourse import bacc

    f16 = mybir.dt.float16
    f32 = mybir.dt.float32
    ACT = mybir.ActivationFunctionType
    ALU = mybir.AluOpType

    nc = bacc.Bacc("TRN2", target_bir_lowering=False, debug=False,
                   enable_asserts=False, num_devices=NCORES)

    xt_d = nc.dram_tensor("xt", (128, KT, TOK), f16, kind="ExternalInput")
    whh_d = nc.dram_tensor("whh", (MT, KT, 128, 128), f16, kind="ExternalInput")
    wxh_d = nc.dram_tensor("wxh", (MT, KT, 128, 128), f16, kind="ExternalInput")
    wxl_d = nc.dram_tensor("wxl", (MT, KT, 128, 128), f16, kind="ExternalInput")
    h0t_d = nc.dram_tensor("h0t", (128, KT, BL), f16, kind="ExternalInput")
    c0t_d = nc.dram_tensor("c0t", (128, KT, BL), f32, kind="ExternalInput")
    biasm_d = nc.dram_tensor("biasm", (128, MT), f32, kind="ExternalInput")
    peep_d = nc.dram_tensor("peep", (128, 3, KT, BL), f32, kind="ExternalInput")
    out_d = nc.dram_tensor("out", (s_steps, 128, KT, BL), f16,
                           kind="ExternalOutput")

    n_blocks = (s_steps + 7) // 8

    with tile.TileContext(nc) as tc:
        with (
            tc.tile_pool(name="wpool", bufs=1) as wpool,
            tc.tile_pool(name="cons", bufs=1) as cons,
            tc.tile_pool(name="xin", bufs=1) as xin,
            tc.tile_pool(name="wxs", bufs=2) as wxs,
            tc.tile_pool(name="ev", bufs=3) as ev,
            tc.tile_pool(name="stg", bufs=2) as stg,
            tc.tile_pool(name="gt", bufs=4) as gt,
            tc.tile_pool(name="st", bufs=3) as st,
            tc.tile_pool(name="ps", bufs=1, space="PSUM") as ps,
            tc.tile_pool(name="pb", bufs=2, space="PSUM") as pb,
            tc.tile_pool(name="dxg", bufs=1, space="DRAM") as dxg,
        ):
            # ---- persistent weights / constants ----
            whh = wpool.tile([128, MT, KT, 128], f16, tag="whh")
            nc.sync.dma_start(whh[:], whh_d[:].rearrange("m k p c -> p m k c"))
            biasm = cons.tile([128, MT], f32, tag="biasm")
            nc.sync.dma_start(biasm[:], biasm_d[:])
            peep = cons.tile([128, 3, KT, BL], f32, tag="peep")
            nc.sync.dma_start(peep[:], peep_d[:])

            xg_d = dxg.tile([MT, 128, TOK], f32, tag="xg")

            # ---- phase A: xg = Wx @ x (+bias), all steps ----
            tok_total = s_steps * BL
            for p0 in range(0, tok_total, 1024):
                pw = min(1024, tok_total - p0)
                chunks = [(c0, min(512, pw - c0)) for c0 in range(0, pw, 512)]
                xtp = xin.tile([128, KT, pw], f16, tag="xtp")
                nc.sync.dma_start(xtp[:], xt_d[:, :, p0:p0 + pw])
                for m in range(MT):
                    wxh_t = wxs.tile([128, KT, 128], f16, tag="wxh")
                    nc.sync.dma_start(
                        wxh_t[:], wxh_d[m].rearrange("k p c -> p k c"))
                    wxl_t = wxs.tile([128, KT, 128], f16, tag="wxl")
                    nc.sync.dma_start(
                        wxl_t[:], wxl_d[m].rearrange("k p c -> p k c"))
                    pch = []
                    pcl = []
                    for ci in range(len(chunks)):
                        pht = ps.tile([128, 512], f32,
                                      tag=("pAh" if ci == 0 else "pBh"),
                                      name=f"ph{ci}")
                        plt = ps.tile([128, 512], f32,
                                      tag=("pAl" if ci == 0 else "pBl"),
                                      name=f"pl{ci}")
                        pch.append(pht)
                        pcl.append(plt)
                    for kt in range(KT):
                        for ci, (c0, cw) in enumerate(chunks):
                            nc.tensor.matmul(
                                pch[ci][:, 0:cw], wxh_t[:, kt, :],
                                xtp[:, kt, c0:c0 + cw],
                                start=(kt == 0), stop=(kt == KT - 1))
                    for kt in range(KT):
                        for ci, (c0, cw) in enumerate(chunks):
                            nc.tensor.matmul(
                                pcl[ci][:, 0:cw], wxl_t[:, kt, :],
                                xtp[:, kt, c0:c0 + cw],
                                start=(kt == 0), stop=(kt == KT - 1))
                    for ci, (c0, cw) in enumerate(chunks):
                        tl = ev.tile([128, 512], f32, tag="tl")
                        nc.vector.tensor_scalar(
                            tl[:, 0:cw], pcl[ci][:, 0:cw], 1.0 / SHIFT,
                            biasm[:, m:m + 1], op0=ALU.mult, op1=ALU.add)
                        evt = ev.tile([128, 512], f32, tag="evt")
                        nc.vector.tensor_add(evt[:, 0:cw], pch[ci][:, 0:cw],
                                             tl[:, 0:cw])
                        nc.sync.dma_start(
                            xg_d[m, :, p0 + c0:p0 + c0 + cw], evt[:, 0:cw])

            # ---- initial state ----
            hT = st.tile([128, KT, BL], f16, tag="hT")
            nc.sync.dma_start(hT[:], h0t_d[:])
            cT = st.tile([128, KT, BL], f32, tag="cT")
            nc.sync.dma_start(cT[:], c0t_d[:])

            # ---- phase B: recurrence ----
            stage = stg.tile([128, MT, 64], f32, tag="stage")
            nc.sync.dma_start(stage[:],
                              xg_d[:, :, 0:64].rearrange("m p t -> p m t"))

            stage_next = None
            for t in range(s_steps):
                if t % 8 == 0:
                    if t > 0:
                        stage = stage_next
                    if t // 8 + 1 < n_blocks:
                        b0 = (t // 8 + 1) * 64
                        stage_next = stg.tile([128, MT, 64], f32, tag="stage")
                        nc.sync.dma_start(
                            stage_next[:],
                            xg_d[:, :, b0:b0 + 64].rearrange("m p t -> p m t"))
                so = (t % 8) * BL

                hT_new = st.tile([128, KT, BL], f16, tag="hT")
                cT_new = st.tile([128, KT, BL], f32, tag="cT")

                # Two-pass contraction: kt 0-3 -> phA, kt 4-7 -> phB, each
                # with consecutive per-m accumulation groups. Pass 1 reads
                # only hT[:, 0:4, :] (ct-half-0 channels), so it can start
                # while the previous step's half-1 gate tail is finishing.
                phA = pb.tile([128, MT, BL], f32, tag="pbA")
                phB = pb.tile([128, MT, BL], f32, tag="pbB")
                for m in range(MT):
                    for kt in range(4):
                        nc.tensor.matmul(
                            phA[:, m, :], whh[:, m, kt, :], hT[:, kt, :],
                            start=(kt == 0), stop=(kt == 3))
                for m in range(MT):
                    for kt in range(4, KT):
                        nc.tensor.matmul(
                            phB[:, m, :], whh[:, m, kt, :], hT[:, kt, :],
                            start=(kt == 4), stop=(kt == KT - 1))

                for half in range(2):
                    h4 = slice(half * 4, half * 4 + 4)
                    m16 = slice(half * 16, half * 16 + 16)
                    # acc = phA + xg + phB   -> [128, 4g, 4ct, 8b]
                    acc = gt.tile([128, 16, BL], f32, tag="acc",
                                  name=f"acc{half}")
                    nc.vector.tensor_add(
                        acc[:], phA[:, m16, :], stage[:, m16, so:so + BL])
                    nc.vector.tensor_add(acc[:], acc[:], phB[:, m16, :])
                    # i/f gates with peephole on c_prev
                    pi = gt.tile([128, 4, BL], f32, tag="pi")
                    nc.vector.tensor_mul(pi[:], peep[:, 0, h4, :], cT[:, h4, :])
                    nc.vector.tensor_add(acc[:, 0:4], acc[:, 0:4], pi[:])
                    pf = gt.tile([128, 4, BL], f32, tag="pf")
                    nc.vector.tensor_mul(pf[:], peep[:, 1, h4, :], cT[:, h4, :])
                    nc.vector.tensor_add(acc[:, 4:8], acc[:, 4:8], pf[:])
                    sif = gt.tile([128, 8, BL], f32, tag="sif")
                    nc.scalar.activation(sif[:], acc[:, 0:8], ACT.Sigmoid)
                    ctl = gt.tile([128, 4, BL], f32, tag="ctl")
                    nc.scalar.activation(ctl[:], acc[:, 8:12], ACT.Tanh)
                    # c_new = f*c + i + tanh(pre_c)
                    fc = gt.tile([128, 4, BL], f32, tag="fc")
                    nc.vector.tensor_mul(fc[:], sif[:, 4:8], cT[:, h4, :])
                    s1 = gt.tile([128, 4, BL], f32, tag="s1")
                    nc.vector.tensor_add(s1[:], sif[:, 0:4], ctl[:])
                    nc.vector.tensor_add(cT_new[:, h4, :], fc[:], s1[:])
                    # o gate with peephole on c_new; h = o * tanh(c_new)
                    po = gt.tile([128, 4, BL], f32, tag="po")
                    nc.vector.tensor_mul(po[:], peep[:, 2, h4, :],
                                         cT_new[:, h4, :])
                    oa = gt.tile([128, 4, BL], f32, tag="oa")
                    nc.vector.tensor_add(oa[:], acc[:, 12:16], po[:])
                    og = gt.tile([128, 4, BL], f32, tag="og")
                    nc.scalar.activation(og[:], oa[:], ACT.Sigmoid)
                    th = gt.tile([128, 4, BL], f32, tag="th")
                    nc.scalar.activation(th[:], cT_new[:, h4, :], ACT.Tanh)
                    nc.vector.tensor_mul(hT_new[:, h4, :], og[:], th[:])

                nc.sync.dma_start(out_d[t], hT_new[:])
                hT, cT = hT_new, cT_new

    nc.compile()
    return nc


_NC_CACHE = None


def _w_tiles_hi(W):
    """(4,H,H) -> [32, KT, 128, 128] fp16 tiles, m=(half,g,ctl)."""
    hi = np.empty((MT, KT, 128, 128), np.float16)
    for m in range(MT):
        half, g, ctl = m // 16, (m % 16) // 4, m % 4
        ct = half * 4 + ctl
        blk = W[g, ct * 128:(ct + 1) * 128, :]
        arr = np.ascontiguousarray(blk.T).reshape(KT, 128, 128)
        hi[m] = arr.astype(np.float16)
    return hi


def _w_tiles_hilo(W, lo_scale=SHIFT):
    hi = np.empty((MT, KT, 128, 128), np.float16)
    lo = np.empty((MT, KT, 128, 128), np.float16)
    for m in range(MT):
        half, g, ctl = m // 16, (m % 16) // 4, m % 4
        ct = half * 4 + ctl
        blk = W[g, ct * 128:(ct + 1) * 128, :]
        arr = np.ascontiguousarray(blk.T).reshape(KT, 128, 128)
        h_ = arr.astype(np.float16)
        hi[m] = h_
        lo[m] = ((arr - h_.astype(np.float32)) * lo_scale).astype(np.float16)
    return hi, lo


def kernel(x, h0, c0, Wx, bx, Wh, bh, peep, bgate):
    global LAST_RESULT, _NC_CACHE
    from concourse import bass_utils

    x = np.asarray(x, dtype=np.float32)
    h0 = np.asarray(h0, dtype=np.float32)
    c0 = np.asarray(c0, dtype=np.float32)
    Wx = np.asarray(Wx, dtype=np.float32)
    Wh = np.asarray(Wh, dtype=np.float32)
    btot = (np.asarray(bx, np.float32) + np.asarray(bh, np.float32)
            + np.asarray(bgate, np.float32))                  # (4, H)
    peep = np.asarray(peep, dtype=np.float32)

    whh = _w_tiles_hi(Wh)
    wxh, wxl = _w_tiles_hilo(Wx)

    biasm = np.empty((128, MT), np.float32)
    for m in range(MT):
        half, g, ctl = m // 16, (m % 16) // 4, m % 4
        ct = half * 4 + ctl
        biasm[:, m] = btot[g, ct * 128:(ct + 1) * 128]
    peep_t = np.ascontiguousarray(
        np.broadcast_to(
            peep.reshape(3, KT, 128).transpose(2, 0, 1)[:, :, :, None],
            (128, 3, KT, BL))).astype(np.float32)

    in_maps = []
    for j in range(NCORES):
        bs = slice(j * BL, (j + 1) * BL)
        xl = x[bs]                                     # (8, S, H)
        xT = np.ascontiguousarray(
            xl.transpose(2, 1, 0).reshape(KT, 128, S, BL)
            .transpose(1, 0, 2, 3).reshape(128, KT, TOK)).astype(np.float16)
        h0T = np.ascontiguousarray(
            h0[bs].T.reshape(KT, 128, BL).transpose(1, 0, 2)).astype(np.float16)
        c0T = np.ascontiguousarray(
            c0[bs].T.reshape(KT, 128, BL).transpose(1, 0, 2)).astype(np.float32)
        in_maps.append({
            "xt": xT, "whh": whh, "wxh": wxh, "wxl": wxl,
            "h0t": h0T, "c0t": c0T, "biasm": biasm,
            "peep": peep_t,
        })

    if _NC_CACHE is None:
        _NC_CACHE = _build_program(S)
    nc = _NC_CACHE

    prev = os.environ.get("BASS_NEVER_TRACE")
    if os.environ.get("KTRACE", "0") != "1":
        os.environ["BASS_NEVER_TRACE"] = "1"
    try:
        try:
            res = bass_utils.run_bass_kernel_spmd(
                nc, in_maps, core_ids=list(range(NCORES)))
        except Exception:
            res = bass_utils.run_bass_kernel_spmd(
                nc, in_maps, core_ids=list(range(NCORES)))
    finally:
        if prev is None:
            os.environ.pop("BASS_NEVER_TRACE", None)
        else:
            os.environ["BASS_NEVER_TRACE"] = prev
    LAST_RESULT = res

    full = np.empty((B, S, H), np.float32)
    for j in range(NCORES):
        r = np.asarray(res.results[j]["out"], dtype=np.float32)
        full[j * BL:(j + 1) * BL] = (
            r.transpose(3, 0, 2, 1).reshape(BL, S, H))
    return np.ascontiguousarray(full)


if __name__ == "__main__":
    import time
    t0 = time.time()
    prog = _build_program(int(os.environ.get("KS", S)))
    print(f"build ok in {time.time() - t0:.1f}s")
